# revision 1
# baseline (speedup 1.0000x reference)
"""CTC loss (keras ctc_batch_cost semantics, blank=C-1) on 8 TRN2 NeuronCores.

Strategy
--------
Data-parallel: 1024 examples sharded 128 per core. Per core:

1. Host prep (numpy, O(B*L), negligible): per-example gather indices for the
   GPSIMD ap_gather ucode op (which shares one index list per 16-partition
   group, so each group is assigned one example) and the CTC skip-mask.
   Index slices are padded to 32-byte stride — ap_gather mis-reads index
   APs whose base byte offset is not 4-byte aligned.
2. Device gather: y_pred staged in [(example,tau) partitions, (tt,class)]
   layout; ap_gather pulls the 48 label classes + blank per timestep;
   an SBUF->SBUF DMA transposes results to [example-partition, time-major].
3. Blank-normalized probability-domain forward DP (no per-step log/exp):
       F'[j] = F[j] + G[j-1]
       G'[i] = ((G[i] + F[i]) + mask[i]*G[i-1]) * (lab_t[i]+eps)/(bl_t+eps)
   with total-mass renormalization every NR steps. F/G/X live in one
   combined state tile so the two adds per step fuse into a single
   two-block DVE instruction; renorm is one reduce over the contiguous
   F|G span.
4. loss = -( ln(F_T[48]+G_T[47]) + sum_t ln(bl_t+eps) + sum_k ln(norm_k) )

Matches the fp32 reference to ~1e-6 relative.

State tile layout [128, 148]:
  cols 0..48   G-block: col0 = zero guard, G_i at col 1+i (i<48)
  cols 49..97  F-block: F_j at col 49+j (j<49)
  cols 98..146 X scratch (col 146 junk)
"""

import numpy as np

B, T, C, L = 1024, 256, 128, 48
NCORES = 8
BC = B // NCORES          # 128 examples per core
NM, NW = 16, 4            # 16 example-batches of 8, 4 time-chunks of 64
TW = T // NW              # 64 timesteps per chunk
KPT = 208                 # gather out elems per partition (16*13), 196 real
GIW = 16                  # int16 cols per batch in gidx (13 used, 32B stride)
EPS = 1e-7
NR = 16                   # renorm period

_CACHED = {}


def _host_gidx(labels_core):
    """[128, NM*GIW] int16 ap_gather indices, one example per 16-part group.

    For batch m, group g (example e=8m+g), out free pos k = tt*49 + j with
    j=0 -> blank(127), j>=1 -> labels[e, j-1]; k in [196,208) padded with 0.
    ap_gather unwraps indices in (s p) order: value for k sits at
    (partition 16g + k%16, slot k//16).
    """
    k = np.arange(KPT)
    tt, j = k // 49, k % 49
    valid = k < 196
    lab = labels_core.reshape(NM, 8, L)  # [m, g, L]
    jl = np.clip(j - 1, 0, L - 1)
    vals = np.where(j[None, None, :] == 0, 127, lab[:, :, jl])
    vals = np.where(valid[None, None, :], vals + 128 * tt[None, None, :], 0)
    gidx = np.zeros((128, NM, GIW), np.int16)
    for m in range(NM):
        for g in range(8):
            gidx[16 * g + (k % 16), m, k // 16] = vals[m, g, :]
    return gidx.reshape(128, NM * GIW)


def _host_mask(labels_core):
    mask = np.zeros((BC, L), np.float32)
    mask[:, 1:] = (labels_core[:, 1:] != labels_core[:, :-1]).astype(np.float32)
    return mask


def _build_nc(debug=False):
    from contextlib import ExitStack
    import concourse.bacc as bacc
    import concourse.tile as tile
    import concourse.mybir as mybir
    from concourse.ap import AP

    f32 = mybir.dt.float32
    Alu = mybir.AluOpType
    Act = mybir.ActivationFunctionType

    nc = bacc.Bacc("TRN2", target_bir_lowering=False, debug=False)
    yD = nc.dram_tensor("y", [BC, T, C], f32, kind="ExternalInput").ap()
    gidxD = nc.dram_tensor("gidx", [128, NM * GIW], mybir.dt.int16,
                           kind="ExternalInput").ap()
    maskD = nc.dram_tensor("mask", [128, L], f32, kind="ExternalInput").ap()
    outD = nc.dram_tensor("out", [BC, 1], f32, kind="ExternalOutput").ap()
    if debug:
        dbg = {
            "dchunk0": nc.dram_tensor("dchunk0", [128, 16 * KPT], f32,
                                      kind="ExternalOutput").ap(),
            "dnorms": nc.dram_tensor("dnorms", [128, 40], f32,
                                     kind="ExternalOutput").ap(),
            "dlnbl": nc.dram_tensor("dlnbl", [128, T], f32,
                                    kind="ExternalOutput").ap(),
            "dfin": nc.dram_tensor("dfin", [128, 1], f32,
                                   kind="ExternalOutput").ap(),
            "dS": nc.dram_tensor("dS", [128, 148], f32,
                                 kind="ExternalOutput").ap(),
        }

    with tile.TileContext(nc) as tc, ExitStack() as ctx:
        cpool = ctx.enter_context(tc.tile_pool(name="const", bufs=1))
        spool = ctx.enter_context(tc.tile_pool(name="state", bufs=1))
        ypool = ctx.enter_context(tc.tile_pool(name="ystage", bufs=24))
        gpool = ctx.enter_context(tc.tile_pool(name="gout", bufs=4))
        kpool = ctx.enter_context(tc.tile_pool(name="chunk", bufs=4))
        rpool = ctx.enter_context(tc.tile_pool(name="rbl", bufs=4))

        gidxT = cpool.tile([128, NM * GIW], mybir.dt.int16)
        nc.sync.dma_start(out=gidxT[:], in_=gidxD)
        maskT = cpool.tile([128, L], f32)
        nc.sync.dma_start(out=maskT[:], in_=maskD)

        Sa = spool.tile([128, 148], f32)
        Sb = spool.tile([128, 148], f32)
        Zt = spool.tile([128, 48], f32)
        norms = spool.tile([128, 40], f32)
        lnblB = spool.tile([128, T], f32)
        rec = spool.tile([128, 1], f32)
        fin = spool.tile([128, 1], f32)
        lnfin = spool.tile([128, 1], f32)
        acc1 = spool.tile([128, 1], f32)
        acc2 = spool.tile([128, 1], f32)
        lossT = spool.tile([128, 1], f32)
        lnnorms = spool.tile([128, 40], f32)

        nc.vector.memset(Sa[:], 0.0)
        nc.vector.memset(Sb[:], 0.0)
        nc.vector.memset(Sa[:, 49:50], 1.0)   # F_0 = 1
        nc.vector.memset(norms[:], 1.0)

        # y[(m e) (w tau tt) c] -> [w, m, e, tau, (tt c)]
        yv = yD.rearrange("(m e) (w tau tt) c -> w m e tau (tt c)",
                          m=NM, e=8, w=NW, tau=16, tt=4)

        def dadd_views(cur, nxt):
            """APs for the fused F'|X double-add.

            out[p,b,k]: b=0 -> F'_k at nxt col 49+k; b=1 -> X_k at col 98+k
            in0[p,b,k] = cur col 49+k (F_k, both blocks)
            in1[p,b,k] = cur col b+k  (b=0: G_{k-1} w/ guard; b=1: G_k)
            """
            out = nxt[:, 49:147].rearrange("p (b k) -> p b k", b=2)
            in0 = cur[:, 49:98].unsqueeze(1).broadcast_to([128, 2, 49])
            base = cur[:, 0:1]
            in1 = AP(base.tensor, base.offset,
                     [list(base.ap[0]), [1, 2], [1, 49]])
            return out, in0, in1

        dpool = ctx.enter_context(tc.tile_pool(name="dscr", bufs=4,
                                               space="DRAM"))

        cur, nxt = Sa, Sb
        kidx = 0
        for w in range(NW):
            chunk = kpool.tile([128, 16 * KPT], f32)
            rblC = rpool.tile([128, TW], f32)
            gob = gpool.tile([128, NM * KPT], f32)
            for m in range(NM):
                st = ypool.tile([128, 512], f32)
                ldeng = nc.sync if m % 2 == 0 else nc.scalar
                ldeng.dma_start(out=st[:], in_=yv[w, m])
                nc.gpsimd.ap_gather(gob[:, KPT * m:KPT * (m + 1)], st[:],
                                    gidxT[:, GIW * m:GIW * m + 13],
                                    channels=128, num_elems=512, d=1,
                                    num_idxs=KPT)
            # relay via DRAM bounce: one SBUF->DRAM writing the transposed
            # layout (DRAM APs take arbitrary dim order), one contiguous
            # DRAM->SBUF. src gob partition 16g+tau, free 208m+k lands at
            # chunk partition 8m+g, free 208tau+k.
            dscr = dpool.tile([128, 16 * KPT], f32)
            db = dscr[:]
            dst = AP(db.tensor, db.offset,
                     [[16 * KPT, 8], [KPT, 16], [8 * 16 * KPT, NM], [1, KPT]])
            nc.scalar.dma_start(out=dst, in_=gob[:])
            nc.scalar.dma_start(out=chunk[:], in_=db)

            # chunk prep: +eps everywhere, then 1/blank and ln(blank)
            nc.vector.tensor_scalar_add(chunk[:], chunk[:], EPS)
            cv = chunk[:].rearrange("p (tau r) -> p tau r", tau=16)
            blankv = cv[:, :, 0:196:49]                     # [128, 16, 4]
            nc.vector.reciprocal(
                rblC[:].rearrange("p (a b) -> p a b", a=16), blankv)
            nc.scalar.activation(
                lnblB[:, TW * w:TW * (w + 1)].rearrange(
                    "p (a b) -> p a b", a=16),
                blankv, Act.Ln)

            t0 = 1 if w == 0 else 0
            if w == 0:
                # t=0 init: G_0 = lab_0[0] * rbl_0 (chunk flat idx 1)
                nc.vector.tensor_mul(Sa[:, 1:2], chunk[:, 1:2], rblC[:, 0:1])
                if debug:
                    nc.sync.dma_start(out=dbg["dchunk0"], in_=chunk[:])

            for tl in range(t0, TW):
                t = TW * w + tl
                base = (tl // 4) * KPT + (tl % 4) * 49
                labp_t = chunk[:, base + 1:base + 49]
                rbl_t = rblC[:, tl:tl + 1]

                out, in0, in1 = dadd_views(cur, nxt)
                nc.vector.tensor_tensor(out, in0, in1, Alu.add)
                nc.vector.tensor_mul(Zt[:], cur[:, 0:48], maskT[:])
                nc.vector.tensor_add(nxt[:, 98:146], nxt[:, 98:146], Zt[:])
                nc.vector.scalar_tensor_tensor(
                    nxt[:, 1:49], nxt[:, 98:146], rbl_t, labp_t,
                    Alu.mult, Alu.mult)
                cur, nxt = nxt, cur

                if t % NR == 0:
                    nc.vector.tensor_reduce(norms[:, kidx:kidx + 1],
                                            cur[:, 0:98],
                                            mybir.AxisListType.X, Alu.add)
                    nc.vector.reciprocal(rec[:], norms[:, kidx:kidx + 1])
                    nc.vector.tensor_scalar_mul(cur[:, 0:98], cur[:, 0:98],
                                                rec[:])
                    kidx += 1

        # final assembly
        if debug:
            nc.sync.dma_start(out=dbg["dnorms"], in_=norms[:])
            nc.sync.dma_start(out=dbg["dlnbl"], in_=lnblB[:])
            nc.sync.dma_start(out=dbg["dS"], in_=cur[:])
        nc.vector.tensor_add(fin[:], cur[:, 97:98], cur[:, 48:49])
        if debug:
            nc.sync.dma_start(out=dbg["dfin"], in_=fin[:])
        nc.scalar.activation(lnfin[:], fin[:], Act.Ln)
        nc.scalar.activation(lnnorms[:], norms[:], Act.Ln)
        nc.vector.tensor_reduce(acc1[:], lnnorms[:], mybir.AxisListType.X,
                                Alu.add)
        nc.vector.tensor_reduce(acc2[:], lnblB[:], mybir.AxisListType.X,
                                Alu.add)
        nc.vector.tensor_add(lossT[:], lnfin[:], acc1[:])
        nc.vector.tensor_add(lossT[:], lossT[:], acc2[:])
        nc.vector.tensor_scalar_mul(lossT[:], lossT[:], -1.0)
        nc.sync.dma_start(out=outD, in_=lossT[:])

    nc.compile()
    return nc


def _get_nc():
    if "nc" not in _CACHED:
        _CACHED["nc"] = _build_nc()
    return _CACHED["nc"]


def make_in_maps(y_pred, labels):
    y_pred = np.ascontiguousarray(np.asarray(y_pred, np.float32))
    labels = np.asarray(labels, np.int32)
    in_maps = []
    for c in range(NCORES):
        sl = slice(BC * c, BC * (c + 1))
        lc = labels[sl]
        in_maps.append({
            "y": np.ascontiguousarray(y_pred[sl]),
            "gidx": _host_gidx(lc),
            "mask": _host_mask(lc),
        })
    return in_maps


def kernel(y_pred, labels):
    from concourse.bass_utils import run_bass_kernel_spmd
    nc = _get_nc()
    in_maps = make_in_maps(y_pred, labels)
    res = run_bass_kernel_spmd(nc, in_maps, list(range(NCORES)))
    return np.concatenate([res.results[c]["out"] for c in range(NCORES)], 0)

